# revision 3
# baseline (speedup 1.0000x reference)
"""MultiHeadAttention (cross-attention, B=32 N=512 L=1024 D=512 H=8) on 8 TRN2 cores.

Strategy: pure data parallelism — each core computes 4 batches end-to-end.
Per-core dataflow (all matmuls in float32r, 1 cycle/row on PE):
  x^T via PE transpose -> Q^T/K^T (+rpb^T) head-major / V natural projections
  scores S^T[l,n] per head with K=64 row-pair packing (tile_position)
  exp via ACT with per-partition mask bias (no max subtraction; scores are O(1))
  stage2 O^T[c,n] = [V|1]^T @ P^T accumulated over l chunks; row 64 = softmax denom
  normalize via reciprocal + gpsimd partition_broadcast, o_proj back to natural
  layout, + bias, DMA out.
"""
import sys

sys.path.insert(0, "/opt/trn_rl_repo")
import numpy as np

B, N, L, D, H, C = 32, 512, 1024, 512, 8, 64
NCORES = 8
BLOC = B // NCORES  # 4 batches per core
SCALE = C ** -0.5
MASK_NEG = -87.0
P = 128
NDC = D // P   # 4 d/e chunks
NLC = L // P   # 8 l chunks
NNC = N // P   # 4 n chunks

_CACHE = {}


def _build_nc():
    import concourse.bacc as bacc
    import concourse.tile as tile
    from concourse import mybir
    from concourse.masks import make_identity

    f32 = mybir.dt.float32
    f32r = mybir.dt.float32r
    u8 = mybir.dt.uint8
    EXP = mybir.ActivationFunctionType.Exp

    nc = bacc.Bacc()
    x_q = nc.declare_dram_parameter("x_q", [BLOC, N, D], f32, isOutput=False)
    x_kv = nc.declare_dram_parameter("x_kv", [BLOC, L, D], f32, isOutput=False)
    pmask = nc.declare_dram_parameter("pad_mask", [BLOC, L], u8, isOutput=False)
    Wq = nc.declare_dram_parameter("Wq", [D, D], f32, isOutput=False)
    Wk = nc.declare_dram_parameter("Wk", [D, D], f32, isOutput=False)
    Wv = nc.declare_dram_parameter("Wv", [D, D], f32, isOutput=False)
    Wo = nc.declare_dram_parameter("Wo", [D, D], f32, isOutput=False)
    bo = nc.declare_dram_parameter("bo", [1, D], f32, isOutput=False)
    rpb = nc.declare_dram_parameter("rpb", [L, D], f32, isOutput=False)
    out = nc.declare_dram_parameter("out", [BLOC, N, D], f32, isOutput=True)

    with tile.TileContext(nc) as tc:
        with (
            tc.tile_pool(name="consts", bufs=1) as consts,
            tc.tile_pool(name="stage", bufs=4) as stage,
            tc.tile_pool(name="xt", bufs=1) as xt_pool,
            tc.tile_pool(name="qkt", bufs=2) as qkt_pool,
            tc.tile_pool(name="vp", bufs=1) as vp_pool,
            tc.tile_pool(name="pt", bufs=6) as pt_pool,
            tc.tile_pool(name="ot", bufs=2) as ot_pool,
            tc.tile_pool(name="outst", bufs=3) as outst_pool,
            tc.tile_pool(name="small", bufs=2) as small,
            tc.tile_pool(name="ps_tp", bufs=2, space="PSUM") as ps_tp,
            tc.tile_pool(name="ps_mm", bufs=4, space="PSUM") as ps_mm,
            tc.tile_pool(name="ps_o", bufs=2, space="PSUM") as ps_o,
        ):
            # ---- one-time setup ----
            warm = consts.tile([P, 1], f32, tag="warm")
            nc.vector.memset(warm, 0.0)
            nc.scalar.activation(out=warm, in_=warm, func=EXP, scale=1.0)

            ident = consts.tile([P, P], f32, tag="ident")
            make_identity(nc, ident)

            ones8 = consts.tile([P, H], f32, tag="ones8")
            nc.vector.memset(ones8, 1.0)

            bo_row = consts.tile([1, D], f32, tag="bo_row")
            nc.sync.dma_start(out=bo_row, in_=bo[:])
            bo_bc = consts.tile([P, D], f32, tag="bo_bc")
            nc.gpsimd.partition_broadcast(bo_bc, bo_row[0:1, :], channels=P)

            # weights -> f32r SBUF tiles, [d_chunk][128, D]
            wsb = {}
            for wi, W in enumerate((Wq, Wk, Wv, Wo)):
                for k in range(NDC):
                    st = stage.tile([P, D], f32, tag="stage")
                    nc.sync.dma_start(out=st, in_=W[k * P:(k + 1) * P, :])
                    wt = consts.tile([P, D], f32r, tag=f"w{wi}_{k}")
                    nc.vector.tensor_copy(wt, st)
                    wsb[(wi, k)] = wt

            # rpb^T [e_chunk][128, L] (f32; added into K^T psum copy)
            rpbT = [consts.tile([P, L], f32, tag=f"rpbT{j}", name=f"rpbT{j}") for j in range(NDC)]
            for i in range(NLC):
                st = stage.tile([P, D], f32, tag="stage")
                nc.sync.dma_start(out=st, in_=rpb[i * P:(i + 1) * P, :])
                for j in range(NDC):
                    pt = ps_tp.tile([P, P], f32, tag="tp")
                    nc.tensor.transpose(pt, st[:, j * P:(j + 1) * P], ident)
                    nc.vector.tensor_copy(rpbT[j][:, i * P:(i + 1) * P], pt)

            # ---- per-batch ----
            for b in range(BLOC):
                # mask bias tile [128, NLC] f32: -87 where pad_mask==1
                tm8 = small.tile([P, NLC], u8, tag="tm8")
                nc.sync.dma_start(
                    out=tm8, in_=pmask[b, :].rearrange("(i p) -> p i", p=P))
                mbias = small.tile([P, NLC], f32, tag="mbias")
                nc.vector.tensor_scalar_mul(mbias, tm8, MASK_NEG)

                # x_q^T tiles [d_chunk][128, N]
                xqT = [xt_pool.tile([P, N], f32r, tag=f"xqT{k}", name=f"xqT{k}") for k in range(NDC)]
                for a in range(NNC):
                    st = stage.tile([P, D], f32, tag="xstage")
                    nc.sync.dma_start(out=st, in_=x_q[b, a * P:(a + 1) * P, :])
                    for k in range(NDC):
                        pt = ps_tp.tile([P, P], f32, tag="tp")
                        nc.tensor.transpose(pt, st[:, k * P:(k + 1) * P], ident)
                        nc.vector.tensor_copy(xqT[k][:, a * P:(a + 1) * P], pt)

                # x_kv^T tiles [d_chunk][128, L]
                xkT = [xt_pool.tile([P, L], f32r, tag=f"xkT{k}", name=f"xkT{k}") for k in range(NDC)]
                for a in range(NLC):
                    st = stage.tile([P, D], f32, tag="xstage")
                    nc.sync.dma_start(out=st, in_=x_kv[b, a * P:(a + 1) * P, :])
                    for k in range(NDC):
                        pt = ps_tp.tile([P, P], f32, tag="tp")
                        nc.tensor.transpose(pt, st[:, k * P:(k + 1) * P], ident)
                        nc.vector.tensor_copy(xkT[k][:, a * P:(a + 1) * P], pt)

                # Q^T [e_chunk][128, N]
                qT = []
                for j in range(NDC):
                    pq = ps_mm.tile([P, N], f32, tag="mm")
                    for k in range(NDC):
                        nc.tensor.matmul(pq, wsb[(0, k)][:, j * P:(j + 1) * P],
                                         xqT[k], start=(k == 0), stop=(k == NDC - 1))
                    t = qkt_pool.tile([P, N], f32r, tag=f"qT{j}")
                    nc.vector.tensor_copy(t, pq)
                    qT.append(t)

                # K^T [e_chunk][128, L], + rpb^T fused into the psum->sbuf add
                kT = []
                for j in range(NDC):
                    t = qkt_pool.tile([P, L], f32r, tag=f"kT{j}")
                    for half in range(2):
                        pk = ps_mm.tile([P, N], f32, tag="mm")
                        for k in range(NDC):
                            nc.tensor.matmul(
                                pk, wsb[(1, k)][:, j * P:(j + 1) * P],
                                xkT[k][:, half * 512:half * 512 + 512],
                                start=(k == 0), stop=(k == NDC - 1))
                        nc.vector.tensor_add(
                            t[:, half * 512:half * 512 + 512], pk,
                            rpbT[j][:, half * 512:half * 512 + 512])
                    kT.append(t)

                # V natural [l_chunk][128, H, C+1] with ones column per head
                vP = []
                for i in range(NLC):
                    pv = ps_mm.tile([P, N], f32, tag="mm")
                    for k in range(NDC):
                        nc.tensor.matmul(pv, xkT[k][:, i * P:(i + 1) * P],
                                         wsb[(2, k)], start=(k == 0),
                                         stop=(k == NDC - 1))
                    t = vp_pool.tile([P, H, C + 1], f32r, tag=f"vp{i}")
                    nc.vector.tensor_copy(
                        t[:, :, 0:C], pv.rearrange("p (h c) -> p h c", h=H))
                    nc.vector.tensor_copy(t[:, :, C:C + 1],
                                          ones8[:, :, None])
                    vP.append(t)

                # attention per head pair
                oT = [ot_pool.tile([P, N], f32r, tag=f"oT{j}", name=f"oT{j}") for j in range(NDC)]
                for j in range(NDC):
                    heads = (2 * j, 2 * j + 1)
                    # scores + exp, all l chunks, both heads
                    ptiles = {h: [] for h in heads}
                    for i in range(NLC):
                        for half, h in enumerate(heads):
                            pss = ps_mm.tile([P, N], f32, tag="mm")
                            lo = 64 * half
                            nc.tensor.matmul(
                                pss, kT[j][lo:lo + 64, i * P:(i + 1) * P],
                                qT[j][lo:lo + 64, :], start=True, stop=True,
                                tile_position=(lo, 0))
                            pe = pt_pool.tile([P, N], f32r, tag="pt")
                            nc.scalar.activation(
                                out=pe, in_=pss, func=EXP,
                                bias=mbias[:, i:i + 1], scale=SCALE)
                            ptiles[h].append(pe)
                    # stage2 + normalize per head
                    for half, h in enumerate(heads):
                        po = ps_o.tile([C + 1, N], f32, tag="st2")
                        for i in range(NLC):
                            nc.tensor.matmul(po, vP[i][:, h, :], ptiles[h][i],
                                             start=(i == 0), stop=(i == NLC - 1))
                        tr = small.tile([1, N], f32, tag="tr")
                        nc.vector.reciprocal(tr, po[C:C + 1, :])
                        trb = small.tile([C, N], f32, tag="trb")
                        nc.gpsimd.partition_broadcast(trb, tr[0:1, :], channels=C)
                        lo = 64 * half
                        nc.vector.tensor_mul(oT[j][lo:lo + 64, :], po[0:C, :], trb)

                # o_proj + bias -> out
                for m in range(NNC):
                    pf = ps_mm.tile([P, N], f32, tag="mm")
                    for j in range(NDC):
                        nc.tensor.matmul(pf, oT[j][:, m * P:(m + 1) * P],
                                         wsb[(3, j)], start=(j == 0),
                                         stop=(j == NDC - 1))
                    to = outst_pool.tile([P, D], f32, tag="outst")
                    nc.vector.tensor_add(to, pf, bo_bc)
                    nc.sync.dma_start(out=out[b, m * P:(m + 1) * P, :], in_=to)

    nc.compile()
    return nc


def _get_nc():
    if "nc" not in _CACHE:
        _CACHE["nc"] = _build_nc()
    return _CACHE["nc"]


def kernel(x_q, x_kv, pad_mask, Wq, Wk, Wv, Wo, bo, rpb):
    from concourse.bass_utils import run_bass_kernel_spmd

    nc = _get_nc()
    x_q = np.asarray(x_q, dtype=np.float32)
    x_kv = np.asarray(x_kv, dtype=np.float32)
    pad_mask = np.asarray(pad_mask).astype(np.uint8)
    shared = {
        "Wq": np.asarray(Wq, np.float32), "Wk": np.asarray(Wk, np.float32),
        "Wv": np.asarray(Wv, np.float32), "Wo": np.asarray(Wo, np.float32),
        "bo": np.asarray(bo, np.float32).reshape(1, D),
        "rpb": np.asarray(rpb, np.float32).reshape(L, D),
    }
    in_maps = []
    for c in range(NCORES):
        sl = slice(c * BLOC, (c + 1) * BLOC)
        in_maps.append({
            "x_q": np.ascontiguousarray(x_q[sl]),
            "x_kv": np.ascontiguousarray(x_kv[sl]),
            "pad_mask": np.ascontiguousarray(pad_mask[sl]),
            **shared,
        })
    res = run_bass_kernel_spmd(nc, in_maps, list(range(NCORES)))
    return np.concatenate([res.results[c]["out"] for c in range(NCORES)], axis=0)
